# revision 6
# baseline (speedup 1.0000x reference)
"""CombinedSparsity (spatial max-pool + lifetime top-k + max-unpool) on 8 TRN2 cores.

Strategy: shard the 128 channels across 8 cores (16 each). Per (b, c) map the
output is all zeros except (possibly) one element: the map's max, written back
at its argmax position, kept only if that max is among the top-6 over the batch
for its channel. The kernel is HBM-read-bound (33.5MB/core), so the stream is
kept saturated and everything else is hidden under it:

  1. stream the shard in 2-channel groups; each group gets a TWO-LEVEL max
     reduce on DVE: HW=4096 -> 64 chunk-partials -> 1 pooled value. Partials
     are spilled to a DRAM scratch tile (64KB/group, on the Act HWDGE queue).
  2. per channel-unit (6/6/4 channels), find the per-channel top-8 batch
     entries (transpose + InstMax/InstMaxIndex), then locate each survivor's
     argmax with two tiny 64-wide lookups: indirect-gather its 64 partials
     (256B) -> chunk index, indirect-gather that 64-elem chunk from x ->
     position in chunk. Scatter the 6 survivors/channel as single f32
     elements. Unit tails overlap the next unit's streaming.
  3. the last channel's DMA is sub-split 4x so only ~1us of reduce plus the
     short find chain is exposed after the final byte lands. Output stays
     zero elsewhere because PJRT output buffers are donated zero-filled.
"""
import numpy as np

import concourse.bass as bass
import concourse.bacc as bacc
import concourse.tile as tile
from concourse import mybir
from concourse.bass_utils import run_bass_kernel_spmd
from concourse.masks import make_identity

B = 128
C_FULL = 128
H = 64
W = 64
HW = H * W
N_CORES = 8
CSH = C_FULL // N_CORES      # channels per core
K = 6                        # lifetime top-k
S = 64                       # chunks per map (two-level reduce)
T = HW // S                  # elems per chunk
F32 = mybir.dt.float32
I32 = mybir.dt.int32
U32 = mybir.dt.uint32

# tail units: (c_lo, c_hi). 6/6/4 so the first two tails hide under streaming.
UNITS = [(0, 6), (6, 12), (12, 16)]
N_SUB = 4                    # sub-DMAs for the final channel

_nc_cache = None


def _build():
    global _nc_cache
    if _nc_cache is not None:
        return _nc_cache

    nc = bacc.Bacc("TRN2", target_bir_lowering=False, debug=False)
    x = nc.dram_tensor("x", [B, CSH, HW], F32, kind="ExternalInput")
    y = nc.dram_tensor("y", [B, CSH, HW], F32, kind="ExternalOutput")
    x64 = x.rearrange("b c (s t) -> (b c s) t", t=T)   # chunk-row view
    y_elem = y.rearrange("b c h -> (b c h)")[:, None]
    n_elem = B * CSH * HW

    with tile.TileContext(nc) as tc:
        with (
            tc.tile_pool(name="const", bufs=1) as cp,
            tc.tile_pool(name="gxp", bufs=5) as gxp,
            tc.tile_pool(name="small", bufs=1) as sp,
            tc.tile_pool(name="dram", bufs=1, space="DRAM") as dp,
            tc.tile_pool(name="ps", bufs=2, space="PSUM") as pp,
        ):
            ident0 = cp.tile([B, B], F32)
            make_identity(nc, ident0[:])
            # keep matmul inputs single-producer-engine (DVE)
            ident = cp.tile([B, B], F32)
            nc.vector.tensor_copy(out=ident[:], in_=ident0[:])

            # channel-index columns per unit (scalar operands must start at
            # partition 0, so a global column can't be partition-sliced)
            c_rel_cols, c_abs_cols = [], []
            for u, (c_lo, c_hi) in enumerate(UNITS):
                ncha = c_hi - c_lo
                rel_i = cp.tile([ncha, 1], I32, name=f"reli{u}")
                nc.gpsimd.iota(
                    rel_i[:], pattern=[[1, 1]], base=0, channel_multiplier=1
                )
                rel_f = cp.tile([ncha, 1], F32, name=f"relf{u}")
                nc.vector.tensor_copy(out=rel_f[:], in_=rel_i[:])
                abs_i = cp.tile([ncha, 1], I32, name=f"absi{u}")
                nc.gpsimd.iota(
                    abs_i[:], pattern=[[1, 1]], base=c_lo, channel_multiplier=1
                )
                abs_f = cp.tile([ncha, 1], F32, name=f"absf{u}")
                nc.vector.tensor_copy(out=abs_f[:], in_=abs_i[:])
                c_rel_cols.append(rel_f)
                c_abs_cols.append(abs_f)

            def emit_unit(u, c_lo, c_hi):
                ncha = c_hi - c_lo
                nsurv = ncha * K

                partials = sp.tile([B, ncha * S], F32, name=f"part{u}")
                pooled = sp.tile([B, ncha], F32, name=f"pooled{u}")
                scratch = dp.tile([B, ncha, S], F32, name=f"scr{u}")

                def load_group(crel, ncols):
                    """Stream ncols channels (unit-relative crel), reduce."""
                    c0 = c_lo + crel
                    gx = gxp.tile([B, ncols * HW], F32, tag="gx")
                    nc.sync.dma_start(out=gx[:], in_=x[:, c0:c0 + ncols, :])
                    nc.vector.tensor_reduce(
                        out=partials[:, crel * S:(crel + ncols) * S],
                        in_=gx[:].rearrange(
                            "p (c s t) -> p c s t", c=ncols, s=S
                        ),
                        axis=mybir.AxisListType.X,
                        op=mybir.AluOpType.max,
                    )
                    finish_group(crel, ncols)

                def finish_group(crel, ncols):
                    nc.vector.tensor_reduce(
                        out=pooled[:, crel:crel + ncols],
                        in_=partials[:, crel * S:(crel + ncols) * S].rearrange(
                            "p (c s) -> p c s", c=ncols
                        ),
                        axis=mybir.AxisListType.X,
                        op=mybir.AluOpType.max,
                    )
                    nc.scalar.dma_start(
                        out=scratch[:, crel:crel + ncols, :],
                        in_=partials[:, crel * S:(crel + ncols) * S],
                    )

                # streaming: 2-channel groups; final channel of the last
                # unit is sub-split so its reduce isn't exposed at the end.
                if u < len(UNITS) - 1:
                    for crel in range(0, ncha, 2):
                        load_group(crel, 2)
                else:
                    for crel in range(0, ncha - 2, 2):
                        load_group(crel, 2)
                    load_group(ncha - 2, 1)
                    crel = ncha - 1
                    c0 = c_lo + crel
                    sub = HW // N_SUB
                    for j in range(N_SUB):
                        gx = gxp.tile([B, sub], F32, tag="gx")
                        nc.sync.dma_start(
                            out=gx[:], in_=x[:, c0:c0 + 1, j * sub:(j + 1) * sub]
                        )
                        nc.vector.tensor_reduce(
                            out=partials[:, crel * S + j * (sub // T):
                                         crel * S + (j + 1) * (sub // T)],
                            in_=gx[:].rearrange("p (s t) -> p s t", t=T),
                            axis=mybir.AxisListType.X,
                            op=mybir.AluOpType.max,
                        )
                    finish_group(crel, 1)

                # ---- unit tail ----
                # per-channel top-8 over the batch
                pooled_t_ps = pp.tile([ncha, B], F32, name=f"ptps{u}")
                nc.tensor.transpose(
                    out=pooled_t_ps[:], in_=pooled[:], identity=ident[:]
                )
                pooled_t = sp.tile([ncha, B], F32, name=f"pt{u}")
                nc.scalar.copy(out=pooled_t[:], in_=pooled_t_ps[:])

                pt8 = sp.tile([ncha, 8], F32, name=f"pt8{u}")
                nc.vector.max(out=pt8[:], in_=pooled_t[:])
                pi8 = sp.tile([ncha, 8], U32, name=f"pi8{u}")
                nc.vector.max_index(
                    out=pi8[:], in_max=pt8[:], in_values=pooled_t[:]
                )
                pi8f = sp.tile([ncha, 8], F32, name=f"pi8f{u}")
                nc.vector.tensor_copy(out=pi8f[:], in_=pi8[:])

                # pack (scratch_row, x_row, value) per candidate, stride-3
                pk = sp.tile([ncha, 8 * 3], F32, name=f"pk{u}")
                pkv = pk[:].rearrange("p (j k) -> p j k", k=3)
                nc.vector.tensor_scalar(
                    out=pkv[:, :, 0:1], in0=pi8f[:], scalar1=float(ncha),
                    scalar2=c_rel_cols[u][:, 0:1],
                    op0=mybir.AluOpType.mult, op1=mybir.AluOpType.add,
                )
                nc.vector.tensor_scalar(
                    out=pkv[:, :, 1:2], in0=pi8f[:], scalar1=float(CSH),
                    scalar2=c_abs_cols[u][:, 0:1],
                    op0=mybir.AluOpType.mult, op1=mybir.AluOpType.add,
                )
                nc.scalar.copy(out=pkv[:, :, 2:3], in_=pt8[:])

                # compact the j<6 survivor slots: [ncha, 6, 3] -> [nsurv, 3]
                cpk = sp.tile([nsurv, 3], F32, name=f"cpk{u}")
                nc.gpsimd.dma_start(out=cpk[:], in_=pkv[:, 0:K, :])

                # needles: each survivor's pooled max, 8-wide
                vb = sp.tile([nsurv, 8], F32, name=f"vb{u}")
                nc.vector.tensor_copy(
                    out=vb[:], in_=cpk[:, 2:3].to_broadcast([nsurv, 8])
                )

                # chunk index via the survivor's 64 partials
                cru_i = sp.tile([nsurv, 1], I32, name=f"cru{u}")
                nc.vector.tensor_copy(out=cru_i[:], in_=cpk[:, 0:1])
                cp64 = sp.tile([nsurv, S], F32, name=f"cp64{u}")
                nc.gpsimd.indirect_dma_start(
                    out=cp64[:], out_offset=None,
                    in_=scratch[:].rearrange("b c s -> (b c) s"),
                    in_offset=bass.IndirectOffsetOnAxis(
                        ap=cru_i[:, 0:1], axis=0
                    ),
                )
                jc8 = sp.tile([nsurv, 8], U32, name=f"jc8{u}")
                nc.vector.max_index(out=jc8[:], in_max=vb[:], in_values=cp64[:])
                jcf = sp.tile([nsurv, 1], F32, name=f"jcf{u}")
                nc.vector.tensor_copy(out=jcf[:], in_=jc8[:, 0:1])

                # position within the chunk via the chunk itself
                rows2 = sp.tile([nsurv, 1], F32, name=f"rows2{u}")
                nc.vector.tensor_scalar(
                    out=rows2[:], in0=cpk[:, 1:2], scalar1=float(S),
                    scalar2=jcf[:, 0:1],
                    op0=mybir.AluOpType.mult, op1=mybir.AluOpType.add,
                )
                rows2_i = sp.tile([nsurv, 1], I32, name=f"rows2i{u}")
                nc.vector.tensor_copy(out=rows2_i[:], in_=rows2[:])
                ck = sp.tile([nsurv, T], F32, name=f"ck{u}")
                nc.gpsimd.indirect_dma_start(
                    out=ck[:], out_offset=None,
                    in_=x64[:],
                    in_offset=bass.IndirectOffsetOnAxis(
                        ap=rows2_i[:, 0:1], axis=0
                    ),
                )
                t8 = sp.tile([nsurv, 8], U32, name=f"t8{u}")
                nc.vector.max_index(out=t8[:], in_max=vb[:], in_values=ck[:])
                tf = sp.tile([nsurv, 1], F32, name=f"tf{u}")
                nc.vector.tensor_copy(out=tf[:], in_=t8[:, 0:1])

                # element offset = (x_row*64 + chunk)*64 + pos, exact in f32
                off_f = sp.tile([nsurv, 1], F32, name=f"off{u}")
                nc.vector.tensor_scalar(
                    out=off_f[:], in0=rows2[:], scalar1=float(T),
                    scalar2=tf[:, 0:1],
                    op0=mybir.AluOpType.mult, op1=mybir.AluOpType.add,
                )
                off_i = sp.tile([nsurv, 1], I32, name=f"offi{u}")
                nc.vector.tensor_copy(out=off_i[:], in_=off_f[:])

                nc.gpsimd.indirect_dma_start(
                    out=y_elem[:],
                    out_offset=bass.IndirectOffsetOnAxis(
                        ap=off_i[:, 0:1], axis=0
                    ),
                    in_=cpk[:, 2:3],
                    in_offset=None,
                    bounds_check=n_elem - 1,
                    oob_is_err=False,
                )

            for u, (c_lo, c_hi) in enumerate(UNITS):
                emit_unit(u, c_lo, c_hi)

    nc.finalize()
    _nc_cache = nc
    return nc


def _install_profile_hook():
    """Inject the antenv.axon_hooks shim so trace=True captures NTFFs."""
    import sys
    import types

    if "antenv.axon_hooks" in sys.modules:
        return
    import antenv
    import trn_agent_boot.trn_boot as tb

    mod = types.ModuleType("antenv.axon_hooks")
    mod._hook = tb._ntff_profile_via_ctypes("/opt/axon/libaxon_pjrt.so")
    mod.get_axon_ntff_profile_hook = lambda: mod._hook
    mod.set_axon_ntff_profile_hook = lambda h: setattr(mod, "_hook", h)
    sys.modules["antenv.axon_hooks"] = mod
    antenv.axon_hooks = mod

    # no S3 in this container — keep artifacts local
    import concourse.bass_utils as bu

    bu.upload_artifacts = lambda tmpdir: tmpdir


def run(activations, trace=False):
    if trace:
        _install_profile_hook()
    act = np.asarray(activations)
    assert act.shape == (B, C_FULL, H, W), act.shape
    act = act.astype(np.float32, copy=False)
    nc = _build()
    in_maps = [
        {"x": np.ascontiguousarray(act[:, i * CSH:(i + 1) * CSH]).reshape(B, CSH, HW)}
        for i in range(N_CORES)
    ]
    res = run_bass_kernel_spmd(
        nc, in_maps, core_ids=list(range(N_CORES)), trace=trace
    )
    out = np.concatenate(
        [r["y"].reshape(B, CSH, H, W) for r in res.results], axis=1
    )
    return out, res


def kernel(activations):
    out, _ = run(activations, trace=False)
    return out
